# revision 1
# baseline (speedup 1.0000x reference)
"""Trainium2 Bass kernel for nn_BEVMultiHistoryCrossAttention.

Deformable-attention style kernel:
  v = value @ W_val.T; off/attn = query @ {W_off,W_attn}.T (+bias);
  aw = softmax; loc = rp + off/128; bilinear-gather v at loc; weighted sum
  over 8 points; mean over 2 histories; out @ W_out.T + b_out + query.

Sharding: 16 (history, head) pairs -> 2 per core (core c: bs=c//4, heads
2*(c%4), 2*(c%4)+1). Host passes value/query pre-transposed [D, Q] so the
projections are direct matmuls (contraction dim on partitions). Each core
projects its value slice, builds a 2x2-block gather table in DRAM (one
256B row per cell holding the 4 bilinear corners hd-major/corner-minor),
gathers one descriptor per (q, head, point) sample via gpsimd dma_gather,
applies bilinear x attention weights on DVE (f16 2x mode: corner index is
the innermost stride-1 dim of data, weights and output) and reduces, then
an AllToAll redistributes per-head partial results by query slice so each
core applies W_out to its own 2048 queries. Host concatenates the 8
output slices.

Self-contained: needs /opt/trn_rl_repo (concourse) + numpy.
"""

import os
import sys
from contextlib import ExitStack

sys.path.insert(0, "/opt/trn_rl_repo")

import ml_dtypes
import numpy as np

import concourse.bass as bass  # noqa: F401
import concourse.bacc as bacc
import concourse.tile as tile
from concourse import mybir

FP = mybir.dt.float32
I16 = mybir.dt.int16
F16 = mybir.dt.float16
ALU = mybir.AluOpType
ACTF = mybir.ActivationFunctionType
AX = mybir.AxisListType

B, R, D = 1, 2, 256
NH, NL, NP, ZA = 8, 1, 8, 4
HD = D // NH  # 32
NCORES = 8

PHASE_MARKS = []  # (phase_label, first_instruction_id) for tlsim analysis


def _mark(label, nc):
    PHASE_MARKS.append((label, int(nc.get_next_instruction_name()[2:])))


class Cfg:
    def __init__(self, H=128, W=128, ch_t=4):
        self.H, self.W = H, W
        self.Q = H * W
        self.T = self.Q // 128          # q-tiles of 128
        self.QS = self.Q // NCORES      # per-core output slice
        self.C = self.T * 16            # idx columns (t, h2, p8)
        self.NIDX = self.C * 128        # total samples per core
        self.CH_T = min(ch_t, self.T)   # q-tiles per gather chunk
        self.CH_COLS = self.CH_T * 16
        self.CH_IDX = self.CH_COLS * 128
        self.NCH = self.T // self.CH_T
        self.GB = min(1024, self.CH_IDX)  # idxs per dma_gather


def build_kernel(cfg: Cfg):
    import concourse.bass_utils as _bu

    if not getattr(_bu, "_dma_scratch_patched", False):
        _orig_gwa = _bu.get_walrus_args

        def _gwa(*a, **k):
            return _orig_gwa(*a, **k) + [
                "--dynamic-dma-scratch-size-per-partition=32768"
            ]

        _bu.get_walrus_args = _gwa
        _bu._dma_scratch_patched = True

    nc = bacc.Bacc(
        "TRN2", target_bir_lowering=False, debug=False,
        num_devices=1 if os.environ.get("KNOCOLL") else NCORES,
        num_swdge_queues=4, dynamic_dma_scratch_size=32768,
    )
    Q = cfg.Q
    io = {}
    io["queryT"] = nc.dram_tensor("queryT", [D, Q], F16, kind="ExternalInput")
    io["valT"] = nc.dram_tensor("valT", [D, Q], F16, kind="ExternalInput")
    io["rp"] = nc.dram_tensor("rp", [Q, ZA, 2], FP, kind="ExternalInput")
    io["wv"] = nc.dram_tensor("wv", [D, 2 * HD], F16, kind="ExternalInput")
    io["bv"] = nc.dram_tensor("bv", [1, 2 * HD], F16, kind="ExternalInput")
    io["wofa"] = nc.dram_tensor("wofa", [D, 48], F16, kind="ExternalInput")
    io["bofa"] = nc.dram_tensor("bofa", [1, 48], F16, kind="ExternalInput")
    io["wout"] = nc.dram_tensor("wout", [4, 128, D], F16, kind="ExternalInput")
    io["bout"] = nc.dram_tensor("bout", [1, D], F16, kind="ExternalInput")
    io["resid"] = nc.dram_tensor("resid", [cfg.QS, D], FP, kind="ExternalInput")
    io["ident"] = nc.dram_tensor("ident", [128, 128], FP, kind="ExternalInput")
    io["out_slice"] = nc.dram_tensor(
        "out_slice", [cfg.QS, D], FP, kind="ExternalOutput"
    )

    with tile.TileContext(nc) as tc:
        for _ in range(int(os.environ.get("KITERS", "1"))):
            _body(tc, cfg, io)

    nc.compile()
    return nc


def _stub_out(tc, cfg, io, ld):
    nc = tc.nc
    z = ld.tile([128, D], FP, tag="stubz")
    nc.vector.memset(z[:], 0.0)
    for s in range(cfg.QS // 128):
        nc.sync.dma_start(io["out_slice"].ap()[s * 128:(s + 1) * 128, :], z[:])


def _body(tc: tile.TileContext, cfg: Cfg, io):
    nc = tc.nc
    H, W, Q, T = cfg.H, cfg.W, cfg.Q, cfg.T
    kphase = int(os.environ.get("KPHASE", "99"))

    es = ExitStack()
    with es:
        consts = es.enter_context(tc.tile_pool(name="consts", bufs=1))
        big = es.enter_context(tc.tile_pool(name="big", bufs=1))
        ld = es.enter_context(tc.tile_pool(name="ld", bufs=3))
        ld2 = es.enter_context(tc.tile_pool(name="ld2", bufs=2))
        sc = es.enter_context(tc.tile_pool(name="sc", bufs=1))
        gp = es.enter_context(tc.tile_pool(name="gp", bufs=3))
        ps = es.enter_context(tc.tile_pool(name="ps", bufs=3, space="PSUM"))
        ps1 = es.enter_context(tc.tile_pool(name="ps1", bufs=2, space="PSUM"))
        dram = es.enter_context(tc.tile_pool(name="dram", bufs=1, space="DRAM"))

        _mark("const", nc)
        # ---- constants ----
        id_sb = consts.tile([128, 128], FP)
        nc.sync.dma_start(id_sb[:], io["ident"].ap())
        id16 = consts.tile([128, 128], F16)
        nc.scalar.copy(id16[:], id_sb[:])
        ones = consts.tile([1, 128], FP)
        nc.vector.memset(ones[:], 1.0)
        ones16 = consts.tile([1, 128], F16)
        nc.vector.memset(ones16[:], 1.0)
        wv_sb = consts.tile([128, 2, 2 * HD], F16)
        nc.sync.dma_start(
            wv_sb[:], io["wv"].ap().rearrange("(c k) n -> k c n", k=128)
        )
        bv_sb = consts.tile([1, 2 * HD], F16)
        nc.sync.dma_start(bv_sb[:], io["bv"].ap())
        wofa_sb = consts.tile([128, 2, 48], F16)
        nc.sync.dma_start(
            wofa_sb[:], io["wofa"].ap().rearrange("(c k) n -> k c n", k=128)
        )
        bofa_sb = consts.tile([1, 48], F16)
        nc.sync.dma_start(bofa_sb[:], io["bofa"].ap())

        # ---- DRAM scratch ----
        v2d = dram.tile([2 * Q + 258, HD], F16)
        tab = dram.tile([2 * Q, 4 * HD], F16)
        idxd = dram.tile([16, cfg.NIDX // 16], I16)
        a2a_s = dram.tile([NCORES, 64, cfg.QS], F16)
        a2a_r = dram.tile([NCORES, 64, cfg.QS], F16)

        _mark("B:pad", nc)
        # Phase B: zero pad rows after v2d (read by shifted tab loads).
        zt = ld.tile([128, HD], F16, tag="zero")
        nc.vector.memset(zt[:], 0.0)
        pad0 = 2 * Q
        npad = 258
        while npad > 0:
            n = min(npad, 128)
            nc.sync.dma_start(v2d[pad0:pad0 + n, :], zt[:n, :])
            pad0 += n
            npad -= n

        _mark("A:proj", nc)
        # ============================================================
        # Phase A: projections. Inputs come in transposed ([D, Q]), so
        # each q-tile is a direct matmul: out[q,n] = sum_d xT[d,q] w[d,n]
        # with the two 128-row d-halves accumulated in PSUM, bias via a
        # ones-row matmul. Loads batched NB q-tiles per DMA.
        # ============================================================
        oa = big.tile([128, T, 48], F16, tag="slotB")
        NB = 8

        RB = min(16, T)  # q-tiles per tab interleave block
        nblk = T // RB

        def _tab_block(hh, blk):
            # corner-interleave rows [hh*Q + blk*RB*128 ...] of v2d into tab
            r0 = hh * Q + blk * RB * 128
            ld4 = ld2.tile([128, RB + 1, 2, HD], F16, tag="itl_in")
            itl = ld2.tile([128, RB, HD, 4], F16, tag="itl")
            if W == 128:
                # corners +W/+W+1 are the +0/+1 data one 128-row tile later
                for ci, sh in enumerate((0, 1)):
                    nc.sync.dma_start(
                        ld4[:, :, ci, :],
                        v2d[r0 + sh:r0 + sh + (RB + 1) * 128, :].rearrange(
                            "(t p) h -> p t h", p=128
                        ),
                    )
                for ci in range(4):
                    nc.scalar.copy(
                        itl[:, :, :, ci],
                        ld4[:, ci // 2:ci // 2 + RB, ci % 2, :],
                    )
            else:
                for ci, sh in enumerate((0, 1, W, W + 1)):
                    nc.sync.dma_start(
                        ld4[:, 0:RB, ci % 2, :] if ci < 2 else
                        ld4[:, 1:RB + 1, ci % 2, :],
                        v2d[r0 + sh:r0 + sh + RB * 128, :].rearrange(
                            "(t p) h -> p t h", p=128
                        ),
                    )
                    nc.vector.tensor_copy(
                        itl[:, :, :, ci],
                        ld4[:, 0:RB, ci % 2, :] if ci < 2 else
                        ld4[:, 1:RB + 1, ci % 2, :],
                    )
            nc.sync.dma_start(
                tab[r0:r0 + RB * 128, :].rearrange(
                    "(t p) e -> p t e", p=128
                ),
                itl[:].rearrange("p t h c -> p t (h c)"),
            )

        qv = io["queryT"].ap().rearrange("(c k) (t q) -> k c t q", k=128, q=128)
        vv = io["valT"].ap().rearrange("(c k) (t q) -> k c t q", k=128, q=128)

        def _proj_block(srcv, wmat, bias, n_out, dst, b):
            xt = ld.tile([128, 2, NB, 128], F16, tag="xt")
            nc.sync.dma_start(xt[:], srcv[:, :, b * NB:(b + 1) * NB, :])
            vstb = None
            if dst is None:
                vstb = ld.tile([128, NB, 2 * HD], F16, tag="vstb")
            for t in range(NB):
                pm = ps1.tile([128, n_out], FP, tag="pm")
                for c in range(2):
                    nc.tensor.matmul(
                        pm[:], xt[:, c, t, :], wmat[:, c, :],
                        start=(c == 0), stop=False,
                    )
                nc.tensor.matmul(
                    pm[:], ones16[:], bias[:], start=False, stop=True
                )
                if dst is not None:
                    nc.scalar.copy(dst[:, b * NB + t, :], pm[:])
                else:
                    nc.scalar.copy(vstb[:, t, :], pm[:])
            if dst is None:
                for hh in range(2):
                    nc.sync.dma_start(
                        v2d[hh * Q + b * NB * 128:
                            hh * Q + (b + 1) * NB * 128, :].rearrange(
                            "(t p) h -> p t h", p=128
                        ),
                        vstb[:, :, hh * HD:(hh + 1) * HD],
                    )

        # interleave query/value projection blocks: query feeds phase D
        # early, value feeds the gather table early
        proj_sched = []
        for b in range(T // NB):
            proj_sched.append(("q", b))
            proj_sched.append(("v", b))
        tab_sched = [(hh2, blk2) for blk2 in range(nblk) for hh2 in range(2)]

        if kphase < 2:
            _stub_out(tc, cfg, io, ld)
            return
        _mark("D:coord", nc)
        # ============================================================
        # Phase D: softmax + coords + weights + idx, chunked over TD
        # q-tiles to bound DVE scratch.
        # ============================================================
        w4 = big.tile([128, T, 16, 1, 4], F16, tag="slotC")
        idxf = big.tile([128, T, 16], FP, tag="ia2")
        TD = min(16, T)

        def _d_pass(t0):
            tsl = slice(t0, t0 + TD)
            aw = sc.tile([128, TD, 2, NP], FP, tag="aw")
            rp_sb = sc.tile([128, TD, 1, 1, ZA, 2], FP, tag="rp")
            nc.sync.dma_start(
                rp_sb[:],
                io["rp"].ap()[t0 * 128:(t0 + TD) * 128].rearrange(
                    "(t p) (a one) (x two) -> p t one two a x", p=128,
                    one=1, two=1,
                ),
            )
            att = oa[:, tsl, 32:48].rearrange("p t (h x) -> p t h x", h=2)
            mx = sc.tile([128, TD, 2], FP, tag="sm1")
            nc.vector.tensor_reduce(mx[:], att, axis=AX.X, op=ALU.max)
            nc.vector.tensor_tensor(
                aw[:], att, mx[:].broadcast_to([128, TD, 2, NP]),
                op=ALU.subtract,
            )
            nc.scalar.activation(aw[:], aw[:], ACTF.Exp)
            sm = sc.tile([128, TD, 2], FP, tag="sm2")
            nc.vector.tensor_reduce(sm[:], aw[:], axis=AX.X, op=ALU.add)
            rcp = sc.tile([128, TD, 2], FP, tag="sm3")
            nc.vector.reciprocal(rcp[:], sm[:])
            nc.vector.tensor_scalar(rcp[:], rcp[:], 0.5, None, op0=ALU.mult)
            nc.vector.tensor_tensor(
                aw[:], aw[:], rcp[:].broadcast_to([128, TD, 2, NP]),
                op=ALU.mult,
            )

            # merged coordinate chain: one pass over all (hh, i2, a, xy)
            # (valid because H == W so the scale is uniform)
            shp6 = [128, TD, 2, 2, ZA, 2]
            offs = oa[:, tsl, 0:32].rearrange(
                "p t (h i a x) -> p t h i a x", h=2, i=2, x=2
            )
            xs = sc.tile(shp6, FP, tag="xs")
            # walrus limits ScalarTensorTensor operands to 3D canonical
            # APs, so emit one per (hh, i2) slice
            for hh in range(2):
                for i2 in range(2):
                    nc.vector.scalar_tensor_tensor(
                        xs[:, :, hh, i2], rp_sb[:, :, 0, 0], float(W),
                        offs[:, :, hh, i2], op0=ALU.mult, op1=ALU.add,
                    )
            # x = xs - 0.5 (true coord). floor via round-to-int
            # (magic 2^23 add) then correct: x0f = rnd - [rnd > x].
            xt_ = sc.tile(shp6, FP, tag="xt_")
            nc.vector.tensor_scalar(
                xt_[:], xs[:], -0.5 + 12582912.0, None, op0=ALU.add
            )
            rnd = sc.tile(shp6, FP, tag="rnd")
            nc.vector.tensor_scalar(
                rnd[:], xt_[:], 12582912.0, None, op0=ALU.subtract
            )
            xtr = sc.tile(shp6, FP, tag="xtr")
            nc.vector.tensor_scalar(
                xtr[:], xs[:], 0.5, None, op0=ALU.subtract
            )
            gt_ = sc.tile(shp6, FP, tag="gt_")
            nc.vector.tensor_tensor(gt_[:], rnd[:], xtr[:], op=ALU.is_gt)
            x0f = sc.tile(shp6, FP, tag="x0f")
            nc.vector.tensor_tensor(x0f[:], rnd[:], gt_[:], op=ALU.subtract)
            wx = sc.tile(shp6, FP, tag="wx")
            nc.vector.tensor_tensor(wx[:], xtr[:], x0f[:], op=ALU.subtract)
            xc = sc.tile([128, TD, 2, 2, ZA], FP, tag="xc")
            yc = sc.tile([128, TD, 2, 2, ZA], FP, tag="yc")
            nc.vector.tensor_scalar(
                xc[:], x0f[:, :, :, :, :, 0], 0.0, float(W - 1),
                op0=ALU.max, op1=ALU.min,
            )
            nc.vector.tensor_scalar(
                yc[:], x0f[:, :, :, :, :, 1], 0.0, float(W - 1),
                op0=ALU.max, op1=ALU.min,
            )
            a_m = sc.tile(shp6, FP, tag="am")
            nc.vector.tensor_scalar(a_m[:], xs[:], 0.5, None, op0=ALU.is_ge)
            b_m = sc.tile(shp6, FP, tag="bm")
            nc.vector.tensor_scalar(
                b_m[:], xs[:], float(W) + 0.5, None, op0=ALU.is_lt
            )
            m_m = sc.tile(shp6, FP, tag="mm")
            nc.vector.tensor_scalar(m_m[:], xs[:], -0.5, None, op0=ALU.is_ge)
            b7 = sc.tile(shp6, FP, tag="b7")
            nc.vector.tensor_scalar(
                b7[:], xs[:], float(W) - 0.5, None, op0=ALU.is_lt
            )
            s0 = sc.tile(shp6, FP, tag="s0")
            nc.vector.tensor_scalar(
                s0[:], wx[:], -1.0, 1.0, op0=ALU.mult, op1=ALU.add
            )
            nc.vector.tensor_tensor(s0[:], s0[:], a_m[:], op=ALU.mult)
            nc.vector.tensor_tensor(s0[:], s0[:], b_m[:], op=ALU.mult)
            nc.vector.tensor_tensor(m_m[:], m_m[:], a_m[:], op=ALU.subtract)
            nc.vector.tensor_tensor(m_m[:], m_m[:], wx[:], op=ALU.mult)
            nc.vector.tensor_tensor(s0[:], s0[:], m_m[:], op=ALU.add)
            s1 = sc.tile(shp6, FP, tag="s1")
            nc.vector.tensor_tensor(s1[:], wx[:], a_m[:], op=ALU.mult)
            nc.vector.tensor_tensor(s1[:], s1[:], b7[:], op=ALU.mult)
            sxy = {(0, 0): s0, (0, 1): s1}  # [(xy-dim select via slicing)]
            for hh in range(2):
                aw_h = aw[:, :, hh, :].rearrange("p t (i a) -> p t i a", i=2)
                for cy in range(2):
                    sy = (s0 if cy == 0 else s1)[:, :, hh, :, :, 1]
                    for cx in range(2):
                        sx = (s0 if cx == 0 else s1)[:, :, hh, :, :, 0]
                        c = cy * 2 + cx
                        dst = w4[:, tsl, hh * NP:(hh + 1) * NP, 0, c].rearrange(
                            "p t (i a) -> p t i a", i=2
                        )
                        wtmp = sc.tile([128, TD, 2, ZA], FP, tag="wtmp")
                        nc.vector.tensor_tensor(wtmp[:], sy, sx, op=ALU.mult)
                        nc.vector.tensor_tensor(dst, wtmp[:], aw_h, op=ALU.mult)
                dsti = idxf[:, tsl, hh * 8:(hh + 1) * 8].rearrange(
                    "p t (i a) -> p t i a", i=2
                )
                nc.vector.scalar_tensor_tensor(
                    dsti,
                    yc[:, :, hh],
                    float(W),
                    xc[:, :, hh],
                    op0=ALU.mult, op1=ALU.add,
                )
                # +0.25 biases into the truncation basin (coords carry
                # ~1e-4 float error); head-1 rows sit Q further down.
                nc.vector.tensor_scalar(
                    idxf[:, tsl, hh * 8:(hh + 1) * 8],
                    idxf[:, tsl, hh * 8:(hh + 1) * 8],
                    0.25 + float(Q) * hh, None, op0=ALU.add,
                )

        # ============================================================
        # Phase E: idx wrap: [128=ql, C] fp32 -> [16=ql%16, c*8+ql//16]
        # int16 via double PE-transpose; DRAM bounce for 8x replication.
        # ============================================================
        if kphase < 3:
            _stub_out(tc, cfg, io, ld)
            return
        idxn = idxf[:].rearrange("p t c -> p (t c)")

        def _e_chunk(ch):
            pt1 = ps.tile([128, 128], FP, tag="tp")
            nc.tensor.transpose(
                pt1[:], idxn[:, ch * 128:(ch + 1) * 128], id_sb[:]
            )
            t1sb = ld2.tile([128, 128], FP, tag="t1sb")
            nc.scalar.copy(t1sb[:], pt1[:])
            iws = ld2.tile([16, 128, 8], I16, tag="iws")
            for qlh in range(8):
                pt2 = ps.tile([16, 128], FP, tag="tp")
                nc.tensor.transpose(
                    pt2[:], t1sb[:, qlh * 16:(qlh + 1) * 16], id_sb[:]
                )
                nc.scalar.copy(iws[:, :, qlh], pt2[:])
            nc.sync.dma_start(
                idxd[:, ch * 1024:(ch + 1) * 1024],
                iws[:].rearrange("p c r -> p (c r)"),
            )

        # fused schedule: projections, D passes, E chunks, tab blocks
        # interleaved so no engine queue is head-of-line blocked.
        qb_done = 0
        d_done = 0
        e_done = 0
        n_epc = cfg.C // 128
        vb_done = 0
        tab_done = 0
        for step, (kind, b) in enumerate(proj_sched):
            if kind == "q":
                _proj_block(qv, wofa_sb, bofa_sb, 48, oa, b)
                qb_done += 1
            else:
                _proj_block(vv, wv_sb, bv_sb, 2 * HD, None, b)
                vb_done += 1
            while (d_done + 1) * TD <= qb_done * NB and d_done < T // TD:
                if kphase >= 2:
                    _d_pass(d_done * TD)
                d_done += 1
                while (e_done + 1) * 8 <= d_done * TD and e_done < n_epc:
                    if kphase >= 3:
                        _e_chunk(e_done)
                    e_done += 1
            while (tab_done < len(tab_sched)
                   and (tab_sched[tab_done][1] + 1) * RB * 128 + 258
                   <= vb_done * NB * 128):
                _tab_block(*tab_sched[tab_done])
                tab_done += 1
        while tab_done < len(tab_sched):
            _tab_block(*tab_sched[tab_done])
            tab_done += 1
        while d_done < T // TD:
            if kphase >= 2:
                _d_pass(d_done * TD)
            d_done += 1
        while e_done < n_epc:
            if kphase >= 3:
                _e_chunk(e_done)
            e_done += 1

        if kphase < 4:
            _stub_out(tc, cfg, io, ld)
            return
        _mark("F:gather", nc)
        # ============================================================
        # Phase F: gather + weighted reduce (chunked over q-tiles).
        # gt free layout per sample is [hd32, c4] so the weight multiply
        # runs in DVE 2x mode (stride-1 f16 innermost on all operands,
        # hd broadcast on a middle dim of the weight AP).
        # ============================================================
        acc = big.tile([128, T, 2, HD], F16, tag="ia2")
        fpc = cfg.CH_IDX // 16  # idx free elems per chunk (per partition)
        GB = cfg.GB
        for g in range(cfg.NCH):
            idxr = gp.tile([128, fpc], I16, tag="idxr")
            nc.sync.dma_start(
                idxr[:],
                idxd[:, g * fpc:(g + 1) * fpc].broadcast_to(
                    [16, fpc, 8]
                ).rearrange("g f r -> r g f"),
            )
            gt = gp.tile([128, cfg.CH_COLS, 128], F16, tag="gath")
            for j in range(cfg.CH_IDX // GB):
                nc.gpsimd.dma_gather(
                    gt[:, j * (GB // 128):(j + 1) * (GB // 128), :],
                    tab[:],
                    idxr[:, j * (GB // 16):(j + 1) * (GB // 16)],
                    num_idxs=GB,
                    num_idxs_reg=GB,
                    elem_size=128,
                    queue_num=(g * (cfg.CH_IDX // GB) + j) % 4,
                )
            t0 = g * cfg.CH_T
            gv = gt[:].rearrange(
                "p (t c) (h k) -> p t c h k", t=cfg.CH_T, h=HD
            )
            red_in = gt[:].rearrange(
                "p (t h c) (d k) -> p t h d c k", t=cfg.CH_T, h=2, k=4
            )
            HT = max(1, cfg.CH_T // 2)  # t-tiles per compute slice
            for hb in range(cfg.CH_T // HT):
                ts0 = hb * HT
                wv4 = w4[:, t0 + ts0:t0 + ts0 + HT, :, :, :].broadcast_to(
                    [128, HT, 16, HD, 4]
                )
                nc.vector.tensor_tensor(
                    gv[:, ts0:ts0 + HT], gv[:, ts0:ts0 + HT], wv4,
                    op=ALU.mult,
                )
                with nc.allow_low_precision(reason="f16 weighted accumulate"):
                    nc.vector.tensor_reduce(
                        acc[:, t0 + ts0:t0 + ts0 + HT, :, :],
                        red_in[:, ts0:ts0 + HT], axis=AX.XY, op=ALU.add,
                    )

        _mark("G:a2a", nc)
        # ============================================================
        # Phase G: acc -> accT -> A2A (stores batched 8 tiles each).
        # ============================================================
        if kphase < 5:
            _stub_out(tc, cfg, io, ld)
            return
        tpd = cfg.QS // 128
        GBT = min(8, tpd)
        gstb = None
        for t in range(T):
            pt = ps.tile([64, 128], F16, tag="tpg")
            nc.tensor.transpose(
                pt[:], acc[:, t, :, :].rearrange("p h d -> p (h d)"), id16[:]
            )
            if t % GBT == 0:
                gstb = ld2.tile([64, GBT, 128], F16, tag="gstb")
            nc.scalar.copy(gstb[:, t % GBT, :], pt[:])
            if t % GBT == GBT - 1:
                d_c = t // tpd
                q0 = ((t % tpd) - (GBT - 1)) * 128
                nc.sync.dma_start(
                    a2a_s[d_c, :, q0:q0 + GBT * 128],
                    gstb[:].rearrange("p t q -> p (t q)"),
                )
        if os.environ.get("KNOCOLL"):
            nc.sync.dma_start(a2a_r[:], a2a_s[:])
        else:
            nc.gpsimd.collective_compute(
                "AllToAll",
                ALU.bypass,
                replica_groups=[list(range(NCORES))],
                ins=[a2a_s[:]],
                outs=[a2a_r[:]],
            )

        _mark("H:out", nc)
        # ============================================================
        # Phase H: final projection on own q-slice + bias + residual.
        # ============================================================
        if kphase < 6:
            _stub_out(tc, cfg, io, ld)
            return
        wout_sb = consts.tile([128, 4, D], F16)
        nc.sync.dma_start(
            wout_sb[:], io["wout"].ap().rearrange("c k n -> k c n")
        )
        bout_sb = consts.tile([1, D], F16)
        nc.sync.dma_start(bout_sb[:], io["bout"].ap())
        a2a_v = a2a_r[:].rearrange("d p q -> (d p) q").rearrange(
            "(c k) (s q) -> k c s q", k=128, q=128
        )
        SBT = min(2, cfg.QS // 128)
        for s0 in range(0, cfg.QS // 128, SBT):
            rsb = ld2.tile([128, 4, SBT, 128], F16, tag="rsbt")
            nc.sync.dma_start(rsb[:], a2a_v[:, :, s0:s0 + SBT, :])
            res_t = ld2.tile([128, SBT, D], FP, tag="rest")
            nc.sync.dma_start(
                res_t[:],
                io["resid"].ap()[s0 * 128:(s0 + SBT) * 128, :].rearrange(
                    "(s p) d -> p s d", p=128
                ),
            )
            fo = ld2.tile([128, SBT, D], FP, tag="fout")
            for si in range(SBT):
                pf = ps1.tile([128, D], FP, tag="pm")
                for c in range(4):
                    nc.tensor.matmul(
                        pf[:], rsb[:, c, si, :], wout_sb[:, c, :],
                        start=(c == 0), stop=False,
                    )
                nc.tensor.matmul(
                    pf[:], ones16[:], bout_sb[:], start=False, stop=True
                )
                nc.vector.tensor_tensor(
                    fo[:, si, :], pf[:], res_t[:, si, :], op=ALU.add
                )
            nc.sync.dma_start(
                io["out_slice"].ap()[s0 * 128:(s0 + SBT) * 128, :].rearrange(
                    "(s p) d -> p s d", p=128
                ),
                fo[:],
            )


def prep_inputs(core, inputs, cfg: Cfg):
    bs = core // 4
    h0 = 2 * (core % 4)
    q = np.ascontiguousarray(np.asarray(inputs["query"], np.float32)[0])
    v = np.asarray(inputs["value"], np.float32)[bs]
    rp_ = np.ascontiguousarray(
        np.asarray(inputs["reference_points"], np.float32)[bs]
    )
    W_val = np.asarray(inputs["W_val"], np.float32)
    b_val = np.asarray(inputs["b_val"], np.float32)
    W_off = np.asarray(inputs["W_off"], np.float32)
    b_off = np.asarray(inputs["b_off"], np.float32)
    W_attn = np.asarray(inputs["W_attn"], np.float32)
    b_attn = np.asarray(inputs["b_attn"], np.float32)
    W_out = np.asarray(inputs["W_out"], np.float32)
    b_out = np.asarray(inputs["b_out"], np.float32)

    wv_h = np.ascontiguousarray(W_val[h0 * HD:(h0 + 2) * HD, :].T)
    bv_h = np.ascontiguousarray(b_val[h0 * HD:(h0 + 2) * HD][None])
    wo_h = W_off[h0 * NP * 2:(h0 + 2) * NP * 2, :].T
    wa_h = W_attn[h0 * NP:(h0 + 2) * NP, :].T
    wofa_h = np.ascontiguousarray(np.concatenate([wo_h, wa_h], axis=1))
    bofa_h = np.ascontiguousarray(
        np.concatenate(
            [b_off[h0 * NP * 2:(h0 + 2) * NP * 2], b_attn[h0 * NP:(h0 + 2) * NP]]
        )[None]
    )
    wout_all = np.ascontiguousarray(
        np.concatenate(
            [W_out[:, (2 * (c % 4)) * HD:(2 * (c % 4) + 2) * HD].T
             for c in range(NCORES)],
            axis=0,
        ).reshape(4, 128, D).astype(np.float16)
    )
    sl = slice(core * cfg.QS, (core + 1) * cfg.QS)
    return {
        "queryT": np.ascontiguousarray(q.T.astype(np.float16)),
        "valT": np.ascontiguousarray(v.T.astype(np.float16)),
        "rp": rp_,
        "wv": wv_h.astype(np.float16),
        "bv": bv_h.astype(np.float16),
        "wofa": wofa_h.astype(np.float16),
        "bofa": bofa_h.astype(np.float16),
        "wout": wout_all,
        "bout": np.ascontiguousarray(b_out[None].astype(np.float16)),
        "resid": np.ascontiguousarray(q[sl]),
        "ident": np.eye(128, dtype=np.float32),
    }


_CACHE = {}


def kernel(**inputs) -> np.ndarray:
    from concourse.bass_utils import run_bass_kernel_spmd

    cfg = Cfg(128, 128)
    if "nc" not in _CACHE:
        _CACHE["nc"] = build_kernel(cfg)
    nc = _CACHE["nc"]
    in_maps = [prep_inputs(c, inputs, cfg) for c in range(NCORES)]
    res = run_bass_kernel_spmd(nc, in_maps, core_ids=list(range(NCORES)))
    out = np.concatenate(
        [res.results[c]["out_slice"] for c in range(NCORES)], axis=0
    )
    return out[None].astype(np.float32)



# revision 16
# speedup vs baseline: 1.0595x; 1.0595x over previous
"""Trainium2 Bass kernel for nn_BEVMultiHistoryCrossAttention.

Deformable-attention style kernel:
  v = value @ W_val.T; off/attn = query @ {W_off,W_attn}.T (+bias);
  aw = softmax; loc = rp + off/128; bilinear-gather v at loc; weighted sum
  over 8 points; mean over 2 histories; out @ W_out.T + b_out + query.

Sharding: 16 (history, head) pairs -> 2 per core (core c: bs=c//4, heads
2*(c%4), 2*(c%4)+1). Host passes value/query pre-transposed [D, Q] so the
projections are direct matmuls (contraction dim on partitions). Each core
projects its value slice, builds a 2x2-block gather table in DRAM (one
256B row per cell holding the 4 bilinear corners hd-major/corner-minor),
gathers one descriptor per (q, head, point) sample via gpsimd dma_gather,
applies bilinear x attention weights on DVE (f16 2x mode: corner index is
the innermost stride-1 dim of data, weights and output) and reduces, then
an AllToAll redistributes per-head partial results by query slice so each
core applies W_out to its own 2048 queries. Host concatenates the 8
output slices.

Self-contained: needs /opt/trn_rl_repo (concourse) + numpy.
"""

import os
import sys
from contextlib import ExitStack

sys.path.insert(0, "/opt/trn_rl_repo")

import ml_dtypes
import numpy as np

import concourse.bass as bass  # noqa: F401
import concourse.bacc as bacc
import concourse.tile as tile
from concourse import mybir

FP = mybir.dt.float32
I16 = mybir.dt.int16
F16 = mybir.dt.float16
F8 = mybir.dt.float8e4
F8NP = ml_dtypes.float8_e4m3
WSCALE = 16.0  # host scales W_off/W_attn/W_val (and biases) by 16 so the
# fp8-quantized weights stay clear of the e4m3 subnormal range; the kernel
# divides offsets by 16, folds 1/16 into the softmax exp, and the host
# divides W_out by 16 to undo the value-path scaling.
ALU = mybir.AluOpType
ACTF = mybir.ActivationFunctionType
AX = mybir.AxisListType

B, R, D = 1, 2, 256
NH, NL, NP, ZA = 8, 1, 8, 4
HD = D // NH  # 32
NCORES = 8

PHASE_MARKS = []  # (phase_label, first_instruction_id) for tlsim analysis


def _mark(label, nc):
    PHASE_MARKS.append((label, int(nc.get_next_instruction_name()[2:])))


class Cfg:
    def __init__(self, H=128, W=128, ch_t=4):
        self.H, self.W = H, W
        self.Q = H * W
        self.T = self.Q // 128          # q-tiles of 128
        self.QS = self.Q // NCORES      # per-core output slice
        self.C = self.T * 16            # idx columns (t, h2, p8)
        self.NIDX = self.C * 128        # total samples per core
        self.CH_T = min(ch_t, self.T)   # q-tiles per gather chunk
        self.CH_COLS = self.CH_T * 16
        self.CH_IDX = self.CH_COLS * 128
        self.NCH = self.T // self.CH_T
        self.GB = min(int(os.environ.get("KGB", "1024")),
                      self.CH_IDX)  # idxs per dma_gather


def build_kernel(cfg: Cfg):
    import concourse.bass_utils as _bu

    if not getattr(_bu, "_dma_scratch_patched", False):
        _orig_gwa = _bu.get_walrus_args

        def _gwa(*a, **k):
            return _orig_gwa(*a, **k) + [
                "--dynamic-dma-scratch-size-per-partition=32768"
            ]

        _bu.get_walrus_args = _gwa
        _bu._dma_scratch_patched = True

    nc = bacc.Bacc(
        "TRN2", target_bir_lowering=False, debug=False,
        num_devices=1 if os.environ.get("KNOCOLL") else NCORES,
        num_swdge_queues=4, dynamic_dma_scratch_size=32768,
    )
    Q = cfg.Q
    io = {}
    io["queryT"] = nc.dram_tensor("queryT", [D, Q], F8, kind="ExternalInput")
    io["valT"] = nc.dram_tensor("valT", [D, Q], F8, kind="ExternalInput")
    io["rp"] = nc.dram_tensor("rp", [Q, ZA, 2], FP, kind="ExternalInput")
    io["wv"] = nc.dram_tensor("wv", [D, 2 * HD], F8, kind="ExternalInput")
    io["bv"] = nc.dram_tensor("bv", [1, 2 * HD], F16, kind="ExternalInput")
    io["wofa"] = nc.dram_tensor("wofa", [D, 48], F8, kind="ExternalInput")
    io["bofa"] = nc.dram_tensor("bofa", [1, 48], F16, kind="ExternalInput")
    io["wout"] = nc.dram_tensor("wout", [4, 128, D], F16, kind="ExternalInput")
    io["bout"] = nc.dram_tensor("bout", [1, D], F16, kind="ExternalInput")
    io["resid"] = nc.dram_tensor("resid", [cfg.QS, D], F16, kind="ExternalInput")
    io["ident"] = nc.dram_tensor("ident", [128, 128], FP, kind="ExternalInput")
    io["out_slice"] = nc.dram_tensor(
        "out_slice", [cfg.QS, D], F16, kind="ExternalOutput"
    )

    with tile.TileContext(nc) as tc:
        for _ in range(int(os.environ.get("KITERS", "1"))):
            _body(tc, cfg, io)

    nc.compile()
    return nc


def _stub_out(tc, cfg, io, ld):
    nc = tc.nc
    z = ld.tile([128, D], F16, tag="stubz")
    nc.vector.memset(z[:], 0.0)
    for s in range(cfg.QS // 128):
        nc.sync.dma_start(io["out_slice"].ap()[s * 128:(s + 1) * 128, :], z[:])


def _body(tc: tile.TileContext, cfg: Cfg, io):
    nc = tc.nc
    H, W, Q, T = cfg.H, cfg.W, cfg.Q, cfg.T
    kphase = int(os.environ.get("KPHASE", "99"))

    es = ExitStack()
    with es:
        consts = es.enter_context(tc.tile_pool(name="consts", bufs=1))
        big = es.enter_context(tc.tile_pool(name="big", bufs=1))
        ld = es.enter_context(tc.tile_pool(name="ld", bufs=3))
        ld2 = es.enter_context(tc.tile_pool(name="ld2", bufs=2))
        sc = es.enter_context(tc.tile_pool(name="sc", bufs=1))
        gp = es.enter_context(tc.tile_pool(name="gp", bufs=3))
        ps = es.enter_context(tc.tile_pool(name="ps", bufs=3, space="PSUM"))
        ps1 = es.enter_context(tc.tile_pool(name="ps1", bufs=2, space="PSUM"))
        dram = es.enter_context(tc.tile_pool(name="dram", bufs=1, space="DRAM"))

        _mark("const", nc)
        # ---- constants ----
        id_sb = consts.tile([128, 128], FP)
        nc.sync.dma_start(id_sb[:], io["ident"].ap())
        id16 = consts.tile([128, 128], F16)
        nc.scalar.copy(id16[:], id_sb[:])
        ones = consts.tile([1, 128], FP)
        nc.vector.memset(ones[:], 1.0)
        ones16 = consts.tile([1, 128], F16)
        nc.vector.memset(ones16[:], 1.0)
        wv_sb = consts.tile([128, 2, 2 * HD], F8)
        nc.sync.dma_start(
            wv_sb[:], io["wv"].ap().rearrange("(c k) n -> k c n", k=128)
        )
        bv_sb = consts.tile([1, 2 * HD], F16)
        nc.sync.dma_start(bv_sb[:], io["bv"].ap())
        wofa_sb = consts.tile([128, 2, 48], F8)
        nc.sync.dma_start(
            wofa_sb[:], io["wofa"].ap().rearrange("(c k) n -> k c n", k=128)
        )
        bofa_sb = consts.tile([1, 48], F16)
        nc.sync.dma_start(bofa_sb[:], io["bofa"].ap())

        # ---- DRAM scratch ----
        v2d = dram.tile([2 * Q + 258, HD], F16)
        tab = dram.tile([2 * Q, 4 * HD], F16)
        idxd = dram.tile([16, cfg.NIDX // 16], I16)
        a2a_s = dram.tile([NCORES, 64, cfg.QS], F16)
        a2a_r = dram.tile([NCORES, 64, cfg.QS], F16)

        _mark("B:pad", nc)
        # Phase B: zero pad rows after v2d (read by shifted tab loads).
        zt = ld.tile([128, HD], F16, tag="zero")
        nc.vector.memset(zt[:], 0.0)
        pad0 = 2 * Q
        npad = 258
        while npad > 0:
            n = min(npad, 128)
            nc.sync.dma_start(v2d[pad0:pad0 + n, :], zt[:n, :])
            pad0 += n
            npad -= n

        _mark("A:proj", nc)
        # ============================================================
        # Phase A: projections. Inputs come in transposed ([D, Q]), so
        # each q-tile is a direct matmul: out[q,n] = sum_d xT[d,q] w[d,n]
        # with the two 128-row d-halves accumulated in PSUM, bias via a
        # ones-row matmul. Loads batched NB q-tiles per DMA.
        # ============================================================
        oa = big.tile([128, T, 48], F16, tag="slotB")
        NB = 8

        RB = min(16, T)  # q-tiles per tab interleave block
        nblk = T // RB

        def _tab_block(hh, blk):
            # corner-interleave rows [hh*Q + blk*RB*128 ...] of v2d into tab.
            # (p t) row order: partition p holds RB consecutive v2d rows, so
            # every DMA run is RB*64B (loads) / RB*256B (store) contiguous —
            # clear of the sub-512B descriptor penalty. tab row i still holds
            # the 2x2 block anchored at cell i.
            r0 = hh * Q + blk * RB * 128
            nrow = RB * 128
            ld4 = ld2.tile([128, 4, RB, HD], F16, tag="itl_in")
            itl = ld2.tile([128, RB, HD, 4], F16, tag="itl")
            for ci, sh in enumerate((0, 1, W, W + 1)):
                nc.sync.dma_start(
                    ld4[:, ci],
                    v2d[r0 + sh:r0 + sh + nrow, :].rearrange(
                        "(p t) h -> p t h", p=128
                    ),
                )
            for ci in range(4):
                nc.scalar.copy(itl[:, :, :, ci], ld4[:, ci])
            nc.sync.dma_start(
                tab[r0:r0 + nrow, :].rearrange(
                    "(p t) e -> p t e", p=128
                ),
                itl[:].rearrange("p t h c -> p t (h c)"),
            )

        qv = io["queryT"].ap().rearrange("(c k) (t q) -> k c t q", k=128, q=128)
        vv = io["valT"].ap().rearrange("(c k) (t q) -> k c t q", k=128, q=128)

        def _proj_block(srcv, wmat, bias, n_out, dst, b):
            xt = ld.tile([128, 2, NB, 128], F8, tag="xt")
            nc.sync.dma_start(xt[:], srcv[:, :, b * NB:(b + 1) * NB, :])
            vstb = None
            if dst is None:
                vstb = ld.tile([128, NB, 2 * HD], F16, tag="vstb")
            for t in range(NB):
                pm = ps1.tile([128, n_out], FP, tag="pm")
                for c in range(2):
                    nc.tensor.matmul(
                        pm[:], xt[:, c, t, :], wmat[:, c, :],
                        start=(c == 0), stop=False,
                    )
                nc.tensor.matmul(
                    pm[:], ones16[:], bias[:], start=False, stop=True
                )
                if dst is not None:
                    nc.scalar.copy(dst[:, b * NB + t, :], pm[:])
                else:
                    nc.scalar.copy(vstb[:, t, :], pm[:])
            if dst is None:
                for hh in range(2):
                    nc.sync.dma_start(
                        v2d[hh * Q + b * NB * 128:
                            hh * Q + (b + 1) * NB * 128, :].rearrange(
                            "(t p) h -> p t h", p=128
                        ),
                        vstb[:, :, hh * HD:(hh + 1) * HD],
                    )

        # interleave query/value projection blocks: query feeds phase D
        # early, value feeds the gather table early
        proj_sched = []
        for b in range(T // NB):
            proj_sched.append(("q", b))
            proj_sched.append(("v", b))
        tab_sched = [(hh2, blk2) for blk2 in range(nblk) for hh2 in range(2)]

        if kphase < 2:
            _stub_out(tc, cfg, io, ld)
            return
        _mark("D:coord", nc)
        # ============================================================
        # Phase D: softmax + coords + weights + idx, chunked over TD
        # q-tiles to bound DVE scratch.
        # ============================================================
        w4 = big.tile([128, T, 16, 1, 4], F16, tag="slotC")
        idxf = big.tile([128, T, 16], FP, tag="ia2")
        TD = min(16, T)

        def _d_pass(t0):
            tsl = slice(t0, t0 + TD)
            aw = sc.tile([128, TD, 2, NP], FP, tag="aw")
            rp_sb = sc.tile([128, TD, 1, 1, ZA, 2], FP, tag="rp")
            nc.sync.dma_start(
                rp_sb[:],
                io["rp"].ap()[t0 * 128:(t0 + TD) * 128].rearrange(
                    "(t p) (a one) (x two) -> p t one two a x", p=128,
                    one=1, two=1,
                ),
            )
            # offsets arrive x16 (fp8 weight rescale) -> descale in place
            nc.vector.tensor_scalar(
                oa[:, tsl, 0:32], oa[:, tsl, 0:32], 1.0 / WSCALE, None,
                op0=ALU.mult,
            )
            att = oa[:, tsl, 32:48].rearrange("p t (h x) -> p t h x", h=2)
            mx = sc.tile([128, TD, 2], FP, tag="sm1")
            nc.vector.tensor_reduce(mx[:], att, axis=AX.X, op=ALU.max)
            nc.vector.tensor_tensor(
                aw[:], att, mx[:].broadcast_to([128, TD, 2, NP]),
                op=ALU.subtract,
            )
            # logits are x16 too: exp((z' - m')/16) == exp(z - m)
            nc.scalar.activation(aw[:], aw[:], ACTF.Exp, scale=1.0 / WSCALE)
            sm = sc.tile([128, TD, 2], FP, tag="sm2")
            nc.vector.tensor_reduce(sm[:], aw[:], axis=AX.X, op=ALU.add)
            rcp = sc.tile([128, TD, 2], FP, tag="sm3")
            nc.vector.reciprocal(rcp[:], sm[:])
            nc.vector.tensor_scalar(rcp[:], rcp[:], 0.5, None, op0=ALU.mult)
            nc.vector.tensor_tensor(
                aw[:], aw[:], rcp[:].broadcast_to([128, TD, 2, NP]),
                op=ALU.mult,
            )

            # merged coordinate chain: one pass over all (hh, i2, a, xy)
            # (valid because H == W so the scale is uniform)
            shp6 = [128, TD, 2, 2, ZA, 2]
            offs = oa[:, tsl, 0:32].rearrange(
                "p t (h i a x) -> p t h i a x", h=2, i=2, x=2
            )
            xs = sc.tile(shp6, FP, tag="xs")
            # walrus limits ScalarTensorTensor operands to 3D canonical
            # APs, so emit one per (hh, i2) slice
            for hh in range(2):
                for i2 in range(2):
                    nc.vector.scalar_tensor_tensor(
                        xs[:, :, hh, i2], rp_sb[:, :, 0, 0], float(W),
                        offs[:, :, hh, i2], op0=ALU.mult, op1=ALU.add,
                    )
            # x = xs - 0.5 (true coord). floor via round-to-int
            # (magic 2^23 add) then correct: x0f = rnd - [rnd > x].
            xt_ = sc.tile(shp6, FP, tag="xt_")
            nc.vector.tensor_scalar(
                xt_[:], xs[:], -0.5 + 12582912.0, None, op0=ALU.add
            )
            rnd = sc.tile(shp6, FP, tag="rnd")
            nc.vector.tensor_scalar(
                rnd[:], xt_[:], 12582912.0, None, op0=ALU.subtract
            )
            xtr = sc.tile(shp6, FP, tag="xtr")
            nc.vector.tensor_scalar(
                xtr[:], xs[:], 0.5, None, op0=ALU.subtract
            )
            gt_ = sc.tile(shp6, FP, tag="gt_")
            nc.vector.tensor_tensor(gt_[:], rnd[:], xtr[:], op=ALU.is_gt)
            x0f = sc.tile(shp6, FP, tag="x0f")
            nc.vector.tensor_tensor(x0f[:], rnd[:], gt_[:], op=ALU.subtract)
            wx = sc.tile(shp6, FP, tag="wx")
            nc.vector.tensor_tensor(wx[:], xtr[:], x0f[:], op=ALU.subtract)
            xc = sc.tile([128, TD, 2, 2, ZA], FP, tag="xc")
            yc = sc.tile([128, TD, 2, 2, ZA], FP, tag="yc")
            nc.vector.tensor_scalar(
                xc[:], x0f[:, :, :, :, :, 0], 0.0, float(W - 1),
                op0=ALU.max, op1=ALU.min,
            )
            nc.vector.tensor_scalar(
                yc[:], x0f[:, :, :, :, :, 1], 0.0, float(W - 1),
                op0=ALU.max, op1=ALU.min,
            )
            a_m = sc.tile(shp6, FP, tag="am")
            nc.vector.tensor_scalar(a_m[:], xs[:], 0.5, None, op0=ALU.is_ge)
            b_m = sc.tile(shp6, FP, tag="bm")
            nc.vector.tensor_scalar(
                b_m[:], xs[:], float(W) + 0.5, None, op0=ALU.is_lt
            )
            m_m = sc.tile(shp6, FP, tag="mm")
            nc.vector.tensor_scalar(m_m[:], xs[:], -0.5, None, op0=ALU.is_ge)
            b7 = sc.tile(shp6, FP, tag="b7")
            nc.vector.tensor_scalar(
                b7[:], xs[:], float(W) - 0.5, None, op0=ALU.is_lt
            )
            s0 = sc.tile(shp6, FP, tag="s0")
            nc.vector.tensor_scalar(
                s0[:], wx[:], -1.0, 1.0, op0=ALU.mult, op1=ALU.add
            )
            nc.vector.tensor_tensor(s0[:], s0[:], a_m[:], op=ALU.mult)
            nc.vector.tensor_tensor(s0[:], s0[:], b_m[:], op=ALU.mult)
            nc.vector.tensor_tensor(m_m[:], m_m[:], a_m[:], op=ALU.subtract)
            nc.vector.tensor_tensor(m_m[:], m_m[:], wx[:], op=ALU.mult)
            nc.vector.tensor_tensor(s0[:], s0[:], m_m[:], op=ALU.add)
            s1 = sc.tile(shp6, FP, tag="s1")
            nc.vector.tensor_tensor(s1[:], wx[:], a_m[:], op=ALU.mult)
            nc.vector.tensor_tensor(s1[:], s1[:], b7[:], op=ALU.mult)
            sxy = {(0, 0): s0, (0, 1): s1}  # [(xy-dim select via slicing)]
            for hh in range(2):
                aw_h = aw[:, :, hh, :].rearrange("p t (i a) -> p t i a", i=2)
                for cy in range(2):
                    sy = (s0 if cy == 0 else s1)[:, :, hh, :, :, 1]
                    for cx in range(2):
                        sx = (s0 if cx == 0 else s1)[:, :, hh, :, :, 0]
                        c = cy * 2 + cx
                        dst = w4[:, tsl, hh * NP:(hh + 1) * NP, 0, c].rearrange(
                            "p t (i a) -> p t i a", i=2
                        )
                        wtmp = sc.tile([128, TD, 2, ZA], FP, tag="wtmp")
                        nc.vector.tensor_tensor(wtmp[:], sy, sx, op=ALU.mult)
                        nc.vector.tensor_tensor(dst, wtmp[:], aw_h, op=ALU.mult)
                dsti = idxf[:, tsl, hh * 8:(hh + 1) * 8].rearrange(
                    "p t (i a) -> p t i a", i=2
                )
                nc.vector.scalar_tensor_tensor(
                    dsti,
                    yc[:, :, hh],
                    float(W),
                    xc[:, :, hh],
                    op0=ALU.mult, op1=ALU.add,
                )
                # +0.25 biases into the truncation basin (coords carry
                # ~1e-4 float error); head-1 rows sit Q further down.
                nc.vector.tensor_scalar(
                    idxf[:, tsl, hh * 8:(hh + 1) * 8],
                    idxf[:, tsl, hh * 8:(hh + 1) * 8],
                    0.25 + float(Q) * hh, None, op0=ALU.add,
                )

        # ============================================================
        # Phase E: idx wrap: [128=ql, C] fp32 -> [16=ql%16, c*8+ql//16]
        # int16 via double PE-transpose; DRAM bounce for 8x replication.
        # ============================================================
        if kphase < 3:
            _stub_out(tc, cfg, io, ld)
            return
        idxn = idxf[:].rearrange("p t c -> p (t c)")

        def _e_chunk(ch):
            pt1 = ps.tile([128, 128], FP, tag="tp")
            nc.tensor.transpose(
                pt1[:], idxn[:, ch * 128:(ch + 1) * 128], id_sb[:]
            )
            t1sb = ld2.tile([128, 128], FP, tag="t1sb")
            nc.scalar.copy(t1sb[:], pt1[:])
            iws = ld2.tile([16, 128, 8], I16, tag="iws")
            for qlh in range(8):
                pt2 = ps.tile([16, 128], FP, tag="tp")
                nc.tensor.transpose(
                    pt2[:], t1sb[:, qlh * 16:(qlh + 1) * 16], id_sb[:]
                )
                nc.scalar.copy(iws[:, :, qlh], pt2[:])
            nc.sync.dma_start(
                idxd[:, ch * 1024:(ch + 1) * 1024],
                iws[:].rearrange("p c r -> p (c r)"),
            )

        # fused schedule: projections, D passes, E chunks, tab blocks
        # interleaved so no engine queue is head-of-line blocked.
        qb_done = 0
        d_done = 0
        e_done = 0
        n_epc = cfg.C // 128
        vb_done = 0
        tab_done = 0
        for step, (kind, b) in enumerate(proj_sched):
            if kind == "q":
                _proj_block(qv, wofa_sb, bofa_sb, 48, oa, b)
                qb_done += 1
            else:
                _proj_block(vv, wv_sb, bv_sb, 2 * HD, None, b)
                vb_done += 1
            while (d_done + 1) * TD <= qb_done * NB and d_done < T // TD:
                if kphase >= 2:
                    _d_pass(d_done * TD)
                d_done += 1
                while (e_done + 1) * 8 <= d_done * TD and e_done < n_epc:
                    if kphase >= 3:
                        _e_chunk(e_done)
                    e_done += 1
            while (tab_done < len(tab_sched)
                   and (tab_sched[tab_done][1] + 1) * RB * 128 + 258
                   <= vb_done * NB * 128):
                _tab_block(*tab_sched[tab_done])
                tab_done += 1
        while tab_done < len(tab_sched):
            _tab_block(*tab_sched[tab_done])
            tab_done += 1
        while d_done < T // TD:
            if kphase >= 2:
                _d_pass(d_done * TD)
            d_done += 1
        while e_done < n_epc:
            if kphase >= 3:
                _e_chunk(e_done)
            e_done += 1

        if kphase < 4:
            _stub_out(tc, cfg, io, ld)
            return
        _mark("F:gather", nc)
        # ============================================================
        # Phase F: gather + weighted reduce (chunked over q-tiles).
        # gt free layout per sample is [hd32, c4] so the weight multiply
        # runs in DVE 2x mode (stride-1 f16 innermost on all operands,
        # hd broadcast on a middle dim of the weight AP).
        # ============================================================
        acc = big.tile([128, T, 2, HD], F16, tag="ia2")
        fpc = cfg.CH_IDX // 16  # idx free elems per chunk (per partition)
        GB = cfg.GB
        for g in range(cfg.NCH):
            idxr = gp.tile([128, fpc], I16, tag="idxr")
            nc.sync.dma_start(
                idxr[:],
                idxd[:, g * fpc:(g + 1) * fpc].broadcast_to(
                    [16, fpc, 8]
                ).rearrange("g f r -> r g f"),
            )
            gt = gp.tile([128, cfg.CH_COLS, 128], F16, tag="gath")
            for j in range(cfg.CH_IDX // GB):
                nc.gpsimd.dma_gather(
                    gt[:, j * (GB // 128):(j + 1) * (GB // 128), :],
                    tab[:],
                    idxr[:, j * (GB // 16):(j + 1) * (GB // 16)],
                    num_idxs=GB,
                    num_idxs_reg=GB,
                    elem_size=128,
                    queue_num=(g * (cfg.CH_IDX // GB) + j) % 4,
                )
            t0 = g * cfg.CH_T
            gv = gt[:].rearrange(
                "p (t c) (h k) -> p t c h k", t=cfg.CH_T, h=HD
            )
            # tree-reduce view: c = 8 points, d = 32 hd, k = 4 corners.
            # TensorReduce runs at 1x on DVE; a TT add-tree runs at 2x
            # (f16 stride-1 last dim), in place inside gt.
            m6 = gt[:].rearrange(
                "p (t h c) (d k) -> p t h c d k", t=cfg.CH_T, h=2, k=4
            )
            HT = max(1, cfg.CH_T // 2)  # t-tiles per compute slice
            for hb in range(cfg.CH_T // HT):
                ts0 = hb * HT
                s = slice(ts0, ts0 + HT)
                wv4 = w4[:, t0 + ts0:t0 + ts0 + HT, :, :, :].broadcast_to(
                    [128, HT, 16, HD, 4]
                )
                nc.vector.tensor_tensor(
                    gv[:, s], gv[:, s], wv4, op=ALU.mult,
                )
                with nc.allow_low_precision(reason="f16 weighted accumulate"):
                    # corner rows (k01 + k23) -> k[2:4]
                    nc.vector.tensor_tensor(
                        m6[:, s, :, :, :, 2:4], m6[:, s, :, :, :, 0:2],
                        m6[:, s, :, :, :, 2:4], op=ALU.add,
                    )
                    # point tree 8 -> 4 -> 2 -> 1, accumulating into c=7
                    nc.vector.tensor_tensor(
                        m6[:, s, :, 4:8, :, 2:4], m6[:, s, :, 0:4, :, 2:4],
                        m6[:, s, :, 4:8, :, 2:4], op=ALU.add,
                    )
                    nc.vector.tensor_tensor(
                        m6[:, s, :, 6:8, :, 2:4], m6[:, s, :, 4:6, :, 2:4],
                        m6[:, s, :, 6:8, :, 2:4], op=ALU.add,
                    )
                    nc.vector.tensor_tensor(
                        m6[:, s, :, 7, :, 2:4], m6[:, s, :, 6, :, 2:4],
                        m6[:, s, :, 7, :, 2:4], op=ALU.add,
                    )
                    # corner cols (k2 + k3) -> acc
                    nc.vector.tensor_tensor(
                        acc[:, t0 + ts0:t0 + ts0 + HT, :, :],
                        m6[:, s, :, 7, :, 2], m6[:, s, :, 7, :, 3],
                        op=ALU.add,
                    )

        _mark("G:a2a", nc)
        # ============================================================
        # Phase G: acc -> accT -> A2A (stores batched 8 tiles each).
        # ============================================================
        if kphase < 5:
            _stub_out(tc, cfg, io, ld)
            return
        tpd = cfg.QS // 128
        GBT = min(8, tpd)
        gstb = None
        for t in range(T):
            pt = ps.tile([64, 128], F16, tag="tpg")
            nc.tensor.transpose(
                pt[:], acc[:, t, :, :].rearrange("p h d -> p (h d)"), id16[:]
            )
            if t % GBT == 0:
                gstb = ld2.tile([64, GBT, 128], F16, tag="gstb")
            nc.scalar.copy(gstb[:, t % GBT, :], pt[:])
            if t % GBT == GBT - 1:
                d_c = t // tpd
                q0 = ((t % tpd) - (GBT - 1)) * 128
                nc.sync.dma_start(
                    a2a_s[d_c, :, q0:q0 + GBT * 128],
                    gstb[:].rearrange("p t q -> p (t q)"),
                )
        if os.environ.get("KNOCOLL"):
            nc.sync.dma_start(a2a_r[:], a2a_s[:])
        else:
            nc.gpsimd.collective_compute(
                "AllToAll",
                ALU.bypass,
                replica_groups=[list(range(NCORES))],
                ins=[a2a_s[:]],
                outs=[a2a_r[:]],
            )

        _mark("H:out", nc)
        # ============================================================
        # Phase H: final projection on own q-slice + bias + residual.
        # ============================================================
        if kphase < 6:
            _stub_out(tc, cfg, io, ld)
            return
        wout_sb = consts.tile([128, 4, D], F16)
        nc.sync.dma_start(
            wout_sb[:], io["wout"].ap().rearrange("c k n -> k c n")
        )
        bout_sb = consts.tile([1, D], F16)
        nc.sync.dma_start(bout_sb[:], io["bout"].ap())
        a2a_v = a2a_r[:].rearrange("d p q -> (d p) q").rearrange(
            "(c k) (s q) -> k c s q", k=128, q=128
        )
        SBT = min(2, cfg.QS // 128)
        for s0 in range(0, cfg.QS // 128, SBT):
            rsb = ld2.tile([128, 4, SBT, 128], F16, tag="rsbt")
            nc.sync.dma_start(rsb[:], a2a_v[:, :, s0:s0 + SBT, :])
            res_t = ld2.tile([128, SBT, D], F16, tag="rest")
            nc.sync.dma_start(
                res_t[:],
                io["resid"].ap()[s0 * 128:(s0 + SBT) * 128, :].rearrange(
                    "(s p) d -> p s d", p=128
                ),
            )
            fo = ld2.tile([128, SBT, D], F16, tag="fout")
            for si in range(SBT):
                pf = ps1.tile([128, D], FP, tag="pm")
                for c in range(4):
                    nc.tensor.matmul(
                        pf[:], rsb[:, c, si, :], wout_sb[:, c, :],
                        start=(c == 0), stop=False,
                    )
                nc.tensor.matmul(
                    pf[:], ones16[:], bout_sb[:], start=False, stop=True
                )
                nc.vector.tensor_tensor(
                    fo[:, si, :], pf[:], res_t[:, si, :], op=ALU.add
                )
            nc.sync.dma_start(
                io["out_slice"].ap()[s0 * 128:(s0 + SBT) * 128, :].rearrange(
                    "(s p) d -> p s d", p=128
                ),
                fo[:],
            )


def prep_inputs(core, inputs, cfg: Cfg):
    bs = core // 4
    h0 = 2 * (core % 4)
    q = np.ascontiguousarray(np.asarray(inputs["query"], np.float32)[0])
    v = np.asarray(inputs["value"], np.float32)[bs]
    rp_ = np.ascontiguousarray(
        np.asarray(inputs["reference_points"], np.float32)[bs]
    )
    W_val = np.asarray(inputs["W_val"], np.float32)
    b_val = np.asarray(inputs["b_val"], np.float32)
    W_off = np.asarray(inputs["W_off"], np.float32)
    b_off = np.asarray(inputs["b_off"], np.float32)
    W_attn = np.asarray(inputs["W_attn"], np.float32)
    b_attn = np.asarray(inputs["b_attn"], np.float32)
    W_out = np.asarray(inputs["W_out"], np.float32)
    b_out = np.asarray(inputs["b_out"], np.float32)

    wv_h = np.ascontiguousarray(W_val[h0 * HD:(h0 + 2) * HD, :].T)
    bv_h = np.ascontiguousarray(b_val[h0 * HD:(h0 + 2) * HD][None])
    wo_h = W_off[h0 * NP * 2:(h0 + 2) * NP * 2, :].T
    wa_h = W_attn[h0 * NP:(h0 + 2) * NP, :].T
    wofa_h = np.ascontiguousarray(np.concatenate([wo_h, wa_h], axis=1))
    bofa_h = np.ascontiguousarray(
        np.concatenate(
            [b_off[h0 * NP * 2:(h0 + 2) * NP * 2], b_attn[h0 * NP:(h0 + 2) * NP]]
        )[None]
    )
    wout_all = np.ascontiguousarray(
        np.concatenate(
            [W_out[:, (2 * (c % 4)) * HD:(2 * (c % 4) + 2) * HD].T
             for c in range(NCORES)],
            axis=0,
        ).reshape(4, 128, D).astype(np.float16)
    )
    sl = slice(core * cfg.QS, (core + 1) * cfg.QS)
    return {
        "queryT": np.ascontiguousarray(q.T.astype(F8NP)),
        "valT": np.ascontiguousarray(v.T.astype(F8NP)),
        "rp": rp_,
        "wv": (WSCALE * wv_h).astype(F8NP),
        "bv": (WSCALE * bv_h).astype(np.float16),
        "wofa": (WSCALE * wofa_h).astype(F8NP),
        "bofa": (WSCALE * bofa_h).astype(np.float16),
        "wout": (wout_all.astype(np.float32) / WSCALE).astype(np.float16),
        "bout": np.ascontiguousarray(b_out[None].astype(np.float16)),
        "resid": np.ascontiguousarray(q[sl].astype(np.float16)),
        "ident": np.eye(128, dtype=np.float32),
    }


_CACHE = {}


def kernel(**inputs) -> np.ndarray:
    from concourse.bass_utils import run_bass_kernel_spmd

    cfg = Cfg(128, 128)
    if "nc" not in _CACHE:
        _CACHE["nc"] = build_kernel(cfg)
    nc = _CACHE["nc"]
    in_maps = [prep_inputs(c, inputs, cfg) for c in range(NCORES)]
    res = run_bass_kernel_spmd(nc, in_maps, core_ids=list(range(NCORES)))
    out = np.concatenate(
        [res.results[c]["out_slice"] for c in range(NCORES)], axis=0
    )
    return out[None].astype(np.float32)

